# revision 20
# baseline (speedup 1.0000x reference)
"""L1 loss (mean |yhat - y|) over (64, 128, 4096) fp32 tensors on 8 TRN2 cores.

Strategy: pure data-parallel over the batch dim; core i takes batch rows
[8i, 8i+8). The host quantizes the inputs (rel-err budget is 2e-2;
measured end-to-end error ~1.5e-3) and interleaves yhat/y per tile into
one DRAM tensor per dtype: 6 tiles in fp8 e4m3 and 2 in bf16. All tile
DMAs are issued up front (the whole per-core working set fits in SBUF),
so DMA completion latency and HBM-contention stragglers surface once,
in the tail, instead of once per tile.

The mixed dtypes balance the two compute engines: the vector engine
computes d = yhat - y per tile (fp8 runs the tensor_tensor at 1x,
~4.35us; bf16 runs 2x_1p, ~2.3us -> 6*4.35 + 2*2.3 = 30.7us total)
while the scalar (ACT) engine does the fused abs+sum via
activation(Abs, accum_out=...) (1x, dtype-independent, ~3.9us/tile =
31.1us total). DMA engines carry 10 MiB/core, ~23us. All three streams
finish within ~1us of each other. (All DVE reduce paths are 1x on HW —
tensor_reduce by spec, tensor_scalar/scalar_tensor_tensor lose their
fast modes when an accumulator output is attached — so a pure-bf16
variant is DMA-bound at 16 MiB and a pure-fp8 variant is DVE-bound at
34.8us; the 6/2 split beats both.) Partials land in fp32 columns of a
[128, 8] accumulator; the host combines them in float64 and divides by
the global element count.
"""

import numpy as np
import ml_dtypes

import concourse.bacc as bacc
import concourse.mybir as mybir
import concourse.tile as tile
from concourse.bass_utils import run_bass_kernel_spmd

N_CORES = 8
FULL_SHAPE = (64, 128, 4096)
TOTAL_ELEMS = FULL_SHAPE[0] * FULL_SHAPE[1] * FULL_SHAPE[2]  # 33,554,432

P = 128                                  # SBUF partitions
ELEMS_PER_CORE = TOTAL_ELEMS // N_CORES  # 4,194,304 per input tensor
F_TILE = 4096                            # elems per partition per tensor per tile
N_TILES = ELEMS_PER_CORE // (P * F_TILE) # 8 tiles
N_BF16 = 3                               # tiles carried as bf16 (processed last)
N_FP8 = N_TILES - N_BF16                 # tiles carried as fp8

_nc_cache = []


def _build_nc():
    nc = bacc.Bacc("TRN2", target_bir_lowering=False, debug=False)
    z8 = nc.declare_dram_parameter(
        "z8", [N_FP8, P, 2 * F_TILE], mybir.dt.float8e4, isOutput=False
    )
    z16 = nc.declare_dram_parameter(
        "z16", [N_BF16, P, 2 * F_TILE], mybir.dt.bfloat16, isOutput=False
    )
    out = nc.declare_dram_parameter(
        "out", [P, N_TILES + 1], mybir.dt.float32, isOutput=True
    )

    with tile.TileContext(nc) as tc:
        with (
            tc.tile_pool(name="io8", bufs=N_FP8) as io8_pool,
            tc.tile_pool(name="io16", bufs=N_BF16) as io16_pool,
            tc.tile_pool(name="wk", bufs=3) as wk_pool,
            tc.tile_pool(name="acc", bufs=1) as acc_pool,
        ):
            acc = acc_pool.tile([P, N_TILES + 1], mybir.dt.float32)
            zts = []
            for i in range(N_FP8):
                zt = io8_pool.tile([P, 2 * F_TILE], mybir.dt.float8e4, tag="z8")
                nc.sync.dma_start(zt[:], z8[i])
                zts.append(zt)
            for i in range(N_BF16):
                zt = io16_pool.tile([P, 2 * F_TILE], mybir.dt.bfloat16, tag="z16")
                nc.sync.dma_start(zt[:], z16[i])
                zts.append(zt)
            for i in range(N_TILES):
                d = wk_pool.tile([P, F_TILE], mybir.dt.bfloat16, tag="d")
                nc.vector.tensor_sub(
                    d[:], zts[i][:, 0:F_TILE], zts[i][:, F_TILE : 2 * F_TILE]
                )
                a = wk_pool.tile([P, F_TILE], mybir.dt.bfloat16, tag="a")
                if i == N_TILES - 1:
                    nc.scalar.activation(
                        a[:, 0:F_TILE // 2],
                        d[:, 0:F_TILE // 2],
                        mybir.ActivationFunctionType.Abs,
                        accum_out=acc[:, i : i + 1],
                    )
                    nc.vector.tensor_reduce(
                        acc[:, i + 1 : i + 2],
                        d[:, F_TILE // 2 : F_TILE],
                        axis=mybir.AxisListType.X,
                        op=mybir.AluOpType.add,
                        apply_absolute_value=True,
                    )
                else:
                    nc.scalar.activation(
                        a[:],
                        d[:],
                        mybir.ActivationFunctionType.Abs,
                        accum_out=acc[:, i : i + 1],
                    )
            nc.sync.dma_start(out[:], acc[:])
    nc.compile()
    return nc


def _get_nc():
    if not _nc_cache:
        _nc_cache.append(_build_nc())
    return _nc_cache[0]


def _shard_inputs(yhat: np.ndarray, y: np.ndarray) -> list[dict[str, np.ndarray]]:
    yhat_t = np.ascontiguousarray(yhat, dtype=np.float32).reshape(
        N_CORES, N_TILES, P, F_TILE
    )
    y_t = np.ascontiguousarray(y, dtype=np.float32).reshape(
        N_CORES, N_TILES, P, F_TILE
    )
    z8 = np.empty((N_CORES, N_FP8, P, 2, F_TILE), dtype=ml_dtypes.float8_e4m3)
    z8[:, :, :, 0, :] = yhat_t[:, :N_FP8]
    z8[:, :, :, 1, :] = y_t[:, :N_FP8]
    z16 = np.empty((N_CORES, N_BF16, P, 2, F_TILE), dtype=ml_dtypes.bfloat16)
    z16[:, :, :, 0, :] = yhat_t[:, N_FP8:]
    z16[:, :, :, 1, :] = y_t[:, N_FP8:]
    z8 = z8.reshape(N_CORES, N_FP8, P, 2 * F_TILE)
    z16 = z16.reshape(N_CORES, N_BF16, P, 2 * F_TILE)
    return [{"z8": z8[c], "z16": z16[c]} for c in range(N_CORES)]


def kernel(yhat: np.ndarray, y: np.ndarray) -> np.ndarray:
    nc = _get_nc()
    in_maps = _shard_inputs(yhat, y)
    res = run_bass_kernel_spmd(nc, in_maps, list(range(N_CORES)))
    total = np.float64(0.0)
    for r in res.results:
        total += r["out"].astype(np.float64).sum()
    return np.asarray(total / TOTAL_ELEMS, dtype=np.float32)


# revision 21
# speedup vs baseline: 1.2147x; 1.2147x over previous
"""L1 loss (mean |yhat - y|) over (64, 128, 4096) fp32 tensors on 8 TRN2 cores.

Strategy: pure data-parallel over the batch dim; core i takes batch rows
[8i, 8i+8). The host quantizes the inputs (rel-err budget is 2e-2;
measured end-to-end error ~1.5e-3) and interleaves yhat/y per tile into
one DRAM tensor per dtype: 6 tiles in fp8 e4m3 and 2 in bf16. All tile
DMAs are issued up front (the whole per-core working set fits in SBUF),
so DMA completion latency and HBM-contention stragglers surface once,
in the tail, instead of once per tile.

The mixed dtypes balance the two compute engines: the vector engine
computes d = yhat - y per tile (fp8 runs the tensor_tensor at 1x,
~4.35us; bf16 runs 2x_1p, ~2.3us -> 6*4.35 + 2*2.3 = 30.7us total)
while the scalar (ACT) engine does the fused abs+sum via
activation(Abs, accum_out=...) (1x, dtype-independent, ~3.9us/tile =
31.1us total). DMA engines carry 10 MiB/core, ~23us. All three streams
finish within ~1us of each other. (All DVE reduce paths are 1x on HW —
tensor_reduce by spec, tensor_scalar/scalar_tensor_tensor lose their
fast modes when an accumulator output is attached — so a pure-bf16
variant is DMA-bound at 16 MiB and a pure-fp8 variant is DVE-bound at
34.8us; the 6/2 split beats both.) Partials land in fp32 columns of a
[128, 8] accumulator; the host combines them in float64 and divides by
the global element count.
"""

import numpy as np
import ml_dtypes

import concourse.bacc as bacc
import concourse.mybir as mybir
import concourse.tile as tile
from concourse.bass_utils import run_bass_kernel_spmd

N_CORES = 8
FULL_SHAPE = (64, 128, 4096)
TOTAL_ELEMS = FULL_SHAPE[0] * FULL_SHAPE[1] * FULL_SHAPE[2]  # 33,554,432

P = 128                                  # SBUF partitions
ELEMS_PER_CORE = TOTAL_ELEMS // N_CORES  # 4,194,304 per input tensor
F_TILE = 4096                            # elems per partition per tensor per tile
N_TILES = ELEMS_PER_CORE // (P * F_TILE) # 8 tiles
N_BF16 = 3                               # tiles carried as bf16 (processed last)
N_FP8 = N_TILES - N_BF16                 # tiles carried as fp8

_nc_cache = []


def _build_nc():
    nc = bacc.Bacc("TRN2", target_bir_lowering=False, debug=False)
    z8 = nc.declare_dram_parameter(
        "z8", [N_FP8, P, 2 * F_TILE], mybir.dt.float8e4, isOutput=False
    )
    z16 = nc.declare_dram_parameter(
        "z16", [N_BF16, P, 2 * F_TILE], mybir.dt.bfloat16, isOutput=False
    )
    out = nc.declare_dram_parameter(
        "out", [P, N_TILES + 1], mybir.dt.float32, isOutput=True
    )

    with tile.TileContext(nc) as tc:
        with (
            tc.tile_pool(name="io8", bufs=N_FP8) as io8_pool,
            tc.tile_pool(name="io16", bufs=N_BF16) as io16_pool,
            tc.tile_pool(name="wk", bufs=2) as wk_pool,
            tc.tile_pool(name="acc", bufs=1) as acc_pool,
        ):
            acc = acc_pool.tile([P, N_TILES + 1], mybir.dt.float32)
            zts = []
            for i in range(N_FP8):
                zt = io8_pool.tile([P, 2 * F_TILE], mybir.dt.float8e4, tag="z8")
                nc.sync.dma_start(zt[:], z8[i])
                zts.append(zt)
            for i in range(N_BF16):
                zt = io16_pool.tile([P, 2 * F_TILE], mybir.dt.bfloat16, tag="z16")
                nc.sync.dma_start(zt[:], z16[i])
                zts.append(zt)
            for i in range(N_TILES):
                d = wk_pool.tile([P, F_TILE], mybir.dt.bfloat16, tag="d")
                nc.vector.tensor_sub(
                    d[:], zts[i][:, 0:F_TILE], zts[i][:, F_TILE : 2 * F_TILE]
                )
                a = wk_pool.tile([P, F_TILE], mybir.dt.bfloat16, tag="a")
                if i == N_TILES - 1:
                    nc.scalar.activation(
                        a[:, 0:F_TILE // 2],
                        d[:, 0:F_TILE // 2],
                        mybir.ActivationFunctionType.Abs,
                        accum_out=acc[:, i : i + 1],
                    )
                    nc.vector.tensor_reduce(
                        acc[:, i + 1 : i + 2],
                        d[:, F_TILE // 2 : F_TILE],
                        axis=mybir.AxisListType.X,
                        op=mybir.AluOpType.add,
                        apply_absolute_value=True,
                    )
                else:
                    nc.scalar.activation(
                        a[:],
                        d[:],
                        mybir.ActivationFunctionType.Abs,
                        accum_out=acc[:, i : i + 1],
                    )
            nc.sync.dma_start(out[:], acc[:])
    nc.compile()
    return nc


def _get_nc():
    if not _nc_cache:
        _nc_cache.append(_build_nc())
    return _nc_cache[0]


def _shard_inputs(yhat: np.ndarray, y: np.ndarray) -> list[dict[str, np.ndarray]]:
    yhat_t = np.ascontiguousarray(yhat, dtype=np.float32).reshape(
        N_CORES, N_TILES, P, F_TILE
    )
    y_t = np.ascontiguousarray(y, dtype=np.float32).reshape(
        N_CORES, N_TILES, P, F_TILE
    )
    z8 = np.empty((N_CORES, N_FP8, P, 2, F_TILE), dtype=ml_dtypes.float8_e4m3)
    z8[:, :, :, 0, :] = yhat_t[:, :N_FP8]
    z8[:, :, :, 1, :] = y_t[:, :N_FP8]
    z16 = np.empty((N_CORES, N_BF16, P, 2, F_TILE), dtype=ml_dtypes.bfloat16)
    z16[:, :, :, 0, :] = yhat_t[:, N_FP8:]
    z16[:, :, :, 1, :] = y_t[:, N_FP8:]
    z8 = z8.reshape(N_CORES, N_FP8, P, 2 * F_TILE)
    z16 = z16.reshape(N_CORES, N_BF16, P, 2 * F_TILE)
    return [{"z8": z8[c], "z16": z16[c]} for c in range(N_CORES)]


def kernel(yhat: np.ndarray, y: np.ndarray) -> np.ndarray:
    nc = _get_nc()
    in_maps = _shard_inputs(yhat, y)
    res = run_bass_kernel_spmd(nc, in_maps, list(range(N_CORES)))
    total = np.float64(0.0)
    for r in res.results:
        total += r["out"].astype(np.float64).sum()
    return np.asarray(total / TOTAL_ELEMS, dtype=np.float32)
